# revision 18
# baseline (speedup 1.0000x reference)
"""Distributed Trainium2 kernel for a 5-layer GCN (PyG GCNConv + BN + ReLU).

Strategy (8 NeuronCores, SPMD single graph):
  - Nodes are permuted (sorted by in-degree, dealt round-robin into 128-node
    tiles) and partitioned core-major: core c owns contiguous device rows.
  - Self-loops are materialized as ordinary edges with weight 1.
  - Per layer: every core holds the full activation table h (node-major rows
    in DRAM, replicated via AllGather with pair-shared outputs).  Edges are
    processed in 128-edge chunks: dma_gather fetches h[src] rows edge-major
    into SBUF, and a streamed block-sparse matrix S (S[e, dst_local] = w_e)
    is matmul'd against the gathered rows on TensorE, accumulating the
    weighted segment-sum directly in PSUM.  GCN symmetric normalization is
    folded in: tables store dinv*h, and the dst-side dinv is applied as a
    per-partition scalar after reduction.
  - Dense W matmul per tile (feature-major), BN statistics via per-partition
    accumulators + a tiny AllReduce, BN+ReLU applied as per-channel
    scale/bias on ScalarE, transpose back to node-major, AllGather.

All index/structure arrays (gather indices, S blocks, schedules) are built
on the host from the graph structure; degree/dinv is host-computed graph
normalization.  All O(E*C) and O(N*C*C) math runs on device.
"""

import math
import os
import sys

os.environ.setdefault("NEURON_SCRATCHPAD_PAGE_SIZE", "2048")  # MB

sys.path.insert(0, "/opt/trn_rl_repo")

import numpy as np
import ml_dtypes

import concourse.bass as bass
import concourse.mybir as mybir
import concourse.bacc as bacc
import concourse.tile as tile
from concourse import bass_utils

NC = 8
TILE = 128
F32 = mybir.dt.float32
BF16 = mybir.dt.bfloat16
I16 = mybir.dt.int16
EPS = 1e-5


# ----------------------------------------------------------------------------
# Host-side planning: permutation, chunk schedule, index/S images per core.
# ----------------------------------------------------------------------------

class Plan:
    pass


def _wrap16(tokens):
    """int16 token list [n*128] -> [128, n*8] image (token i at [i%16, i//16],
    replicated 8x down the partitions for the 8 Q7 cores)."""
    n = tokens.shape[0]
    img16 = np.ascontiguousarray(tokens.reshape(n // 16, 16).T)
    return np.tile(img16, (8, 1))


def build_plan(x, edge_index, edge_weight, widths):
    P = Plan()
    N, C0 = x.shape
    E = edge_index.shape[1]
    P.N, P.C0, P.E = N, C0, E
    ntiles = math.ceil(N / (NC * TILE)) * NC
    P.ntpc = ntiles // NC              # tiles per core
    P.npc = P.ntpc * TILE              # nodes per core
    P.Npad = ntiles * TILE
    P.HALF = (P.Npad // 2 + TILE - 1) // TILE * TILE
    assert P.HALF < 32768 and P.Npad - P.HALF < 32768

    src = np.asarray(edge_index[0], dtype=np.int64)
    dst = np.asarray(edge_index[1], dtype=np.int64)
    ew = np.asarray(edge_weight, dtype=np.float32)

    cnt = np.bincount(dst, minlength=N)
    deg = np.bincount(dst, weights=ew.astype(np.float64), minlength=N).astype(np.float32) + 1.0
    dinv = (1.0 / np.sqrt(deg)).astype(np.float32)

    order = np.argsort(-cnt, kind="stable")          # sorted orig ids, high degree first
    pos = np.arange(P.Npad)
    t, p = pos // TILE, pos % TILE
    dev_of_sorted = (t % NC) * P.npc + (t // NC) * TILE + p
    orig_of_dev = np.full(P.Npad, -1, dtype=np.int64)
    dev_of_orig = np.empty(N, dtype=np.int64)
    orig_of_dev[dev_of_sorted[:N]] = order
    dev_of_orig[order] = dev_of_sorted[:N]
    P.orig_of_dev, P.dev_of_orig, P.dinv = orig_of_dev, dev_of_orig, dinv

    # device-space edges + self loops (weight 1.0)
    sdev = np.concatenate([dev_of_orig[src], dev_of_orig])
    ddev = np.concatenate([dev_of_orig[dst], dev_of_orig])
    wall = np.concatenate([ew, np.ones(N, dtype=np.float32)])

    o = np.argsort(ddev, kind="stable")
    sdev, ddev, wall = sdev[o], ddev[o], wall[o]
    # tile id of each edge (by dst) and boundaries
    tile_of = ddev // TILE
    bounds = np.searchsorted(tile_of, np.arange(ntiles + 1))

    # per (core, slot, half) edge groups
    groups = {}
    nlo = np.ones(P.ntpc, dtype=np.int64)
    nhi = np.ones(P.ntpc, dtype=np.int64)
    for g in range(ntiles):
        # g is a device-tile id: dev = c*npc + k*128 + p  =>  g = c*ntpc + k
        c, k = g // P.ntpc, g % P.ntpc
        lo_, hi_ = bounds[g], bounds[g + 1]
        s_, d_, w_ = sdev[lo_:hi_], ddev[lo_:hi_] % TILE, wall[lo_:hi_]
        m = s_ < P.HALF
        for half, msk in ((0, m), (1, ~m)):
            ss, dd, ww = s_[msk], d_[msk], w_[msk]
            oo = np.argsort(dd, kind="stable")
            groups[(c, k, half)] = (ss[oo], dd[oo], ww[oo])
            n = max(1, math.ceil(len(ss) / TILE))
            if half == 0:
                nlo[k] = max(nlo[k], n)
            else:
                nhi[k] = max(nhi[k], n)
    P.nlo, P.nhi = nlo, nhi
    P.tot_chunks = int(nlo.sum() + nhi.sum())

    # flat idx / S images per core (identical shapes across cores)
    idx_flats, s_flats = [], []
    for c in range(NC):
        idx_parts, s_parts = [], []
        for k in range(P.ntpc):
            for half, nsch in ((0, int(nlo[k])), (1, int(nhi[k]))):
                ss, dd, ww = groups[(c, k, half)]
                ntok = nsch * TILE
                tok = np.zeros(ntok, dtype=np.int16)
                base = P.HALF if half else 0
                tok[: len(ss)] = (ss - base).astype(np.int16)
                idx_parts.append(_wrap16(tok))
                S = np.zeros((nsch, TILE, TILE), dtype=np.float32)
                q = np.arange(len(ss))
                S[q // TILE, q % TILE, dd] = ww
                # SBUF image: [128 edge-partitions, nsch*128 cols]
                img = np.ascontiguousarray(S.transpose(1, 0, 2)).reshape(TILE, -1)
                s_parts.append(np.ascontiguousarray(img).reshape(-1))
        idx_flats.append(np.ascontiguousarray(np.concatenate(idx_parts, axis=1)))
        s_flats.append(np.concatenate(s_parts).astype(ml_dtypes.bfloat16))
    P.idx_flats, P.s_flats = idx_flats, s_flats
    P.idx_total = idx_flats[0].shape[1]
    P.s_total = s_flats[0].shape[0]

    # per-core maskdinv [128, ntpc] (0 at pad nodes), f32
    P.maskdinv = []
    for c in range(NC):
        md = np.zeros((TILE, P.ntpc), dtype=np.float32)
        for k in range(P.ntpc):
            devs = c * P.npc + k * TILE + np.arange(TILE)
            real = orig_of_dev[devs] >= 0
            md[real, k] = dinv[orig_of_dev[devs][real]]
        P.maskdinv.append(md)

    # layer configs: (Cin, Cout, table_dtype_np, table_width)
    # 512B gather elements are ~1.6x faster per descriptor than 256B, so
    # tables for cin<=128 are stored f32 (cin=64 zero-padded to 128 cols);
    # cin=256 tables stay bf16 (already 512B rows).
    P.widths = widths
    dims = [C0] + list(widths)
    P.layers = []
    for li in range(len(widths)):
        cin, cout = dims[li], dims[li + 1]
        if cin <= TILE:
            P.layers.append((cin, cout, np.float32, TILE))
        else:
            P.layers.append((cin, cout, ml_dtypes.bfloat16, cin))

    # h0 table: dinv * x, device order, padded to table width, f32
    h0 = np.zeros((P.Npad, P.layers[0][3]), dtype=np.float32)
    real = orig_of_dev >= 0
    h0[real, :C0] = x[orig_of_dev[real]] * dinv[orig_of_dev[real], None]
    P.h0 = h0.astype(P.layers[0][2])
    return P


# ----------------------------------------------------------------------------
# Graph builder
# ----------------------------------------------------------------------------

def mdt(np_dtype):
    return BF16 if np_dtype == ml_dtypes.bfloat16 else F32


def build_graph(nc, P, weights, gammas, betas):
    ntpc, npc, HALF, Npad = P.ntpc, P.npc, P.HALF, P.Npad
    NREAL = float(P.N)
    ablate = set(os.environ.get("GCN_ABLATE", "").split(","))

    # ---- external inputs -------------------------------------------------
    idx_in = nc.dram_tensor("idx", [TILE, P.idx_total], I16, kind="ExternalInput")
    s_in = nc.dram_tensor("sblk", [P.s_total], BF16, kind="ExternalInput")
    md_in = nc.dram_tensor("maskdinv", [TILE, ntpc], F32, kind="ExternalInput")
    h0_in = nc.dram_tensor("h0", [Npad, P.layers[0][3]], mdt(P.layers[0][2]), kind="ExternalInput")
    ident_in = nc.dram_tensor("ident", [TILE, TILE], BF16, kind="ExternalInput")
    identf_in = nc.dram_tensor("identf", [TILE, TILE], F32, kind="ExternalInput")
    w_ins, g_ins, b_ins = [], [], []
    for li, (cin, cout, _, _tw) in enumerate(P.layers):
        kcs = math.ceil(cin / TILE)
        hvs = math.ceil(cout / TILE)
        w_ins.append(nc.dram_tensor(f"W{li}", [TILE, kcs * cout], F32, kind="ExternalInput"))
        g_ins.append(nc.dram_tensor(f"g{li}", [TILE, hvs], F32, kind="ExternalInput"))
        b_ins.append(nc.dram_tensor(f"bb{li}", [TILE, hvs], F32, kind="ExternalInput"))
    out_t = nc.dram_tensor("out", [npc, P.layers[-1][1]], F32, kind="ExternalOutput")

    # ---- internal DRAM ---------------------------------------------------
    tables = [h0_in]
    shards = []
    for li, (cin, cout, _, _tw) in enumerate(P.layers[:-1]):
        tdt_next = mdt(P.layers[li + 1][2])
        tw_next = P.layers[li + 1][3]
        tables.append(nc.dram_tensor(f"h{li+1}", [Npad, tw_next], tdt_next, kind="Internal",
                                     addr_space="Shared"))
        shards.append(nc.dram_tensor(f"shard{li}", [npc, tw_next], tdt_next, kind="Internal"))
    ar_in = nc.dram_tensor("ar_in", [TILE, 4], F32, kind="Internal")
    ar_out = nc.dram_tensor("ar_out", [TILE, 4], F32, kind="Internal", addr_space="Shared")

    from contextlib import ExitStack
    with tile.TileContext(nc) as tc, ExitStack() as es:
        pool = es.enter_context(tc.tile_pool(name="persist", bufs=1))
        gpool = es.enter_context(tc.tile_pool(name="gather", bufs=2))
        spool = es.enter_context(tc.tile_pool(name="sblk", bufs=2))
        epool = es.enter_context(tc.tile_pool(name="epi", bufs=4))
        ppool = es.enter_context(tc.tile_pool(name="psum", bufs=2, space="PSUM"))
        tpool = es.enter_context(tc.tile_pool(name="tpsum", bufs=2, space="PSUM"))
        ypool = es.enter_context(tc.tile_pool(name="ypsum", bufs=2, space="PSUM"))
        ybpool = es.enter_context(tc.tile_pool(name="ybuf", bufs=1))

        # persistent loads
        idx_sb = pool.tile([TILE, P.idx_total], I16)
        nc.sync.dma_start(idx_sb[:], idx_in[:, :])
        md_sb = pool.tile([TILE, ntpc], F32)
        nc.sync.dma_start(md_sb[:], md_in[:, :])
        ident_sb = pool.tile([TILE, TILE], BF16)
        nc.sync.dma_start(ident_sb[:], ident_in[:, :])
        identf_sb = pool.tile([TILE, TILE], F32)
        nc.sync.dma_start(identf_sb[:], identf_in[:, :])
        w_sb, g_sb, b_sb = [], [], []
        for li, (cin, cout, _, _tw) in enumerate(P.layers):
            kcs = math.ceil(cin / TILE)
            hvs = math.ceil(cout / TILE)
            wt = pool.tile([TILE, kcs * cout], F32, name=f"w{li}sb")
            nc.sync.dma_start(wt[:], w_ins[li][:, :])
            w_sb.append(wt)
            gt = pool.tile([TILE, hvs], F32, name=f"g{li}sb")
            nc.sync.dma_start(gt[:], g_ins[li][:, :])
            g_sb.append(gt)
            bt = pool.tile([TILE, hvs], F32, name=f"b{li}sb")
            nc.sync.dma_start(bt[:], b_ins[li][:, :])
            b_sb.append(bt)

        # idx slice offsets per (k, half)
        idx_off = {}
        s_off = {}
        io = so = 0
        for k in range(ntpc):
            for half, n in ((0, int(P.nlo[k])), (1, int(P.nhi[k]))):
                idx_off[(k, half)] = io
                s_off[(k, half)] = so
                io += n * TILE // 16
                so += n * TILE * TILE

        for li, (cin, cout, tdt_np, twidth) in enumerate(P.layers):
            tdt = mdt(tdt_np)
            table = tables[li]
            elem = twidth
            kcs = math.ceil(cin / TILE)
            hvs = math.ceil(cout / TILE)
            last = li == len(P.layers) - 1
            sdt = tdt  # S matmul dtype matches gathered dtype

            strip_sum = epool.tile([TILE, ntpc * hvs], F32, name=f"ssum{li}", bufs=1)
            strip_sq = epool.tile([TILE, ntpc * hvs], F32, name=f"ssq{li}", bufs=1)
            ybuf = ybpool.tile([TILE, ntpc * hvs * TILE], F32, name=f"ybuf{li}", tag="ybuf")

            for k in range(ntpc):
                psz = ppool.tile([TILE, cin], F32, name=f"psz{li}", tag="psz")
                nch_tot = int(P.nlo[k]) + int(P.nhi[k])
                qglob = 0
                for half in (0, 1):
                    n = int(P.nlo[k]) if half == 0 else int(P.nhi[k])
                    ntok = n * TILE
                    ioff = idx_off[(k, half)]
                    soff = s_off[(k, half)]
                    st = spool.tile([TILE, n * TILE], sdt, name=f"st{li}", tag="sblk")
                    if sdt == BF16:
                        nc.sync.dma_start(
                            st[:], s_in.ap()[soff: soff + TILE * n * TILE]
                            .rearrange("(p c) -> p c", p=TILE))
                    else:
                        # SWDGE cast bf16 -> f32 during load
                        nc.gpsimd.dma_start(
                            st[:], s_in.ap()[soff: soff + TILE * n * TILE]
                            .rearrange("(p c) -> p c", p=TILE))
                    gt = gpool.tile([TILE, n * elem], tdt, name=f"gt{li}", tag="gath")
                    base_ap = table[0:HALF, :] if half == 0 else table[HALF:Npad, :]
                    if "nogather" in ablate:
                        nc.sync.dma_start(
                            gt[:], table[0:ntok, :].rearrange(
                                "(a b) e -> a (b e)", a=TILE))
                    else:
                        nc.gpsimd.dma_gather(
                            gt.rearrange("p (q e) -> p q e", e=elem),
                            base_ap,
                            idx_sb[:, ioff: ioff + ntok // 16],
                            ntok, ntok, elem,
                            single_packet=False,
                        )
                    for q in range(n):
                        if "nope" in ablate and not (qglob == 0 or qglob == nch_tot - 1):
                            qglob += 1
                            continue
                        nc.tensor.matmul(
                            psz[:, :],
                            st[:, q * TILE:(q + 1) * TILE],
                            gt[:, q * elem:q * elem + cin],
                            start=(qglob == 0),
                            stop=True if "nope" in ablate else (qglob == nch_tot - 1),
                        )
                        qglob += 1

                # epilogue: z = psz * maskdinv  -> bf16
                z_sb = epool.tile([TILE, cin], F32, name=f"z{li}", tag="z")
                nc.vector.tensor_scalar(z_sb[:], psz[:], md_sb[:, k:k + 1], None,
                                        mybir.AluOpType.mult)
                # transpose z to feature-major
                zT = []
                for kc in range(kcs):
                    w = min(TILE, cin - kc * TILE)
                    pt = tpool.tile([TILE, TILE], F32, name=f"pzt{li}", tag="tp")
                    nc.tensor.transpose(pt[:w, :TILE], z_sb[:, kc * TILE: kc * TILE + w],
                                        identf_sb[:TILE, :TILE])
                    zt = epool.tile([TILE, TILE], F32, name=f"zt{li}", tag="zt")
                    nc.scalar.copy(zt[:w, :], pt[:w, :])
                    zT.append((zt, w))
                # dense W matmul, feature-major y
                for h in range(hvs):
                    hw = min(TILE, cout - h * TILE)
                    py = ypool.tile([TILE, TILE], F32, name=f"py{li}", tag="py")
                    for kc in range(kcs):
                        zt, w = zT[kc]
                        nc.tensor.matmul(
                            py[:hw, :TILE],
                            w_sb[li][:w, kc * cout + h * TILE: kc * cout + h * TILE + hw],
                            zt[:w, :TILE],
                            start=(kc == 0), stop=(kc == kcs - 1),
                        )
                    col = k * hvs + h
                    # park y (bf16) + per-channel sums
                    nc.vector.tensor_scalar(
                        ybuf[:hw, col * TILE:(col + 1) * TILE], py[:hw, :], 1.0, 0.0,
                        mybir.AluOpType.mult, mybir.AluOpType.add,
                        accum_out=strip_sum[:hw, col:col + 1])
                    sq = epool.tile([TILE, TILE], F32, name=f"sq{li}", tag="sq")
                    nc.scalar.activation(sq[:hw, :], py[:hw, :],
                                         mybir.ActivationFunctionType.Square,
                                         accum_out=strip_sq[:hw, col:col + 1])

            # ---- BN stats: reduce strips, AllReduce, scale/bias ----
            pack = epool.tile([TILE, 4], F32, name=f"pack{li}", tag="pack")
            nc.gpsimd.memset(pack[:], 0.0)
            for h in range(hvs):
                hw = min(TILE, cout - h * TILE)
                nc.vector.tensor_reduce(
                    pack[:hw, h:h + 1],
                    strip_sum[:hw, h::hvs] if hvs > 1 else strip_sum[:hw, :],
                    mybir.AxisListType.X, mybir.AluOpType.add)
                nc.vector.tensor_reduce(
                    pack[:hw, 2 + h:3 + h],
                    strip_sq[:hw, h::hvs] if hvs > 1 else strip_sq[:hw, :],
                    mybir.AxisListType.X, mybir.AluOpType.add)
            nc.sync.dma_start(ar_in[:, :], pack[:])
            if "noag" not in ablate:
                nc.gpsimd.collective_compute(
                    "AllReduce", mybir.AluOpType.add,
                    replica_groups=[list(range(NC))],
                    ins=[ar_in.ap()], outs=[ar_out.ap()])
            arr = epool.tile([TILE, 4], F32, name=f"arr{li}", tag="arr")
            nc.sync.dma_start(arr[:], (ar_out if "noag" not in ablate else ar_in)[:, :])
            # m = sum/NREAL ; v = sq/NREAL - m^2 ; scale = g*rsqrt(v+eps); bias = bb - m*scale
            mvec = epool.tile([TILE, 2], F32, name=f"m{li}", tag="mv")
            nc.vector.tensor_scalar(mvec[:, 0:2], arr[:, 0:2], 1.0 / NREAL, None,
                                    mybir.AluOpType.mult)
            vvec = epool.tile([TILE, 2], F32, name=f"v{li}", tag="vv")
            nc.vector.tensor_scalar(vvec[:, 0:2], arr[:, 2:4], 1.0 / NREAL, None,
                                    mybir.AluOpType.mult)
            msq = epool.tile([TILE, 2], F32, name=f"msq{li}", tag="msq")
            nc.vector.tensor_tensor(msq[:, :], mvec[:, :], mvec[:, :], mybir.AluOpType.mult)
            nc.vector.tensor_tensor(vvec[:, :], vvec[:, :], msq[:, :], mybir.AluOpType.subtract)
            nc.vector.tensor_scalar(vvec[:, :], vvec[:, :], EPS, None, mybir.AluOpType.add)
            sqr = epool.tile([TILE, 2], F32, name=f"sqr{li}", tag="sqr")
            nc.scalar.activation(sqr[:, :], vvec[:, :], mybir.ActivationFunctionType.Sqrt)
            rin = epool.tile([TILE, 2], F32, name=f"rin{li}", tag="rin")
            nc.vector.reciprocal(rin[:, :], sqr[:, :])
            scl = epool.tile([TILE, 2], F32, name=f"scl{li}", tag="scl")
            nc.vector.tensor_tensor(scl[:, 0:hvs], rin[:, 0:hvs], g_sb[li][:, 0:hvs],
                                    mybir.AluOpType.mult)
            bia = epool.tile([TILE, 2], F32, name=f"bia{li}", tag="bia")
            nc.vector.tensor_tensor(bia[:, 0:hvs], mvec[:, 0:hvs], scl[:, 0:hvs],
                                    mybir.AluOpType.mult)
            nc.vector.tensor_tensor(bia[:, 0:hvs], b_sb[li][:, 0:hvs], bia[:, 0:hvs],
                                    mybir.AluOpType.subtract)

            # ---- BN apply + transpose-out + write shard ----
            tdt_next = mdt(P.layers[li + 1][2]) if not last else F32
            tw_next = P.layers[li + 1][3] if not last else cout
            for k in range(ntpc):
                stage = epool.tile([TILE, tw_next], tdt_next, name=f"stg{li}", tag="stage")
                if tw_next > cout:
                    nc.vector.memset(stage[:, cout:tw_next], 0.0)
                for h in range(hvs):
                    hw = min(TILE, cout - h * TILE)
                    col = k * hvs + h
                    bn = epool.tile([TILE, TILE], BF16 if not last else F32,
                                    name=f"bn{li}", tag="bn")
                    if not last:
                        nc.scalar.activation(
                            bn[:hw, :], ybuf[:hw, col * TILE:(col + 1) * TILE],
                            mybir.ActivationFunctionType.Relu,
                            bias=bia[:hw, h:h + 1], scale=scl[:hw, h:h + 1])
                    else:
                        nc.vector.tensor_scalar(
                            bn[:hw, :], ybuf[:hw, col * TILE:(col + 1) * TILE],
                            scl[:hw, h:h + 1], bia[:hw, h:h + 1],
                            mybir.AluOpType.mult, mybir.AluOpType.add)
                    pt2 = tpool.tile([TILE, TILE], BF16 if not last else F32,
                                     name=f"pt2{li}", tag="tp")
                    nc.tensor.transpose(pt2[:TILE, :hw], bn[:hw, :TILE],
                                        (ident_sb if not last else identf_sb)[:hw, :hw])
                    if not last:
                        nc.vector.tensor_scalar(
                            stage[:, h * TILE: h * TILE + hw], pt2[:, :hw],
                            md_sb[:, k:k + 1], None, mybir.AluOpType.mult)
                    else:
                        nc.vector.tensor_copy(stage[:, h * TILE: h * TILE + hw],
                                              pt2[:, :hw])
                dst_ap = (shards[li] if not last else out_t)[k * TILE:(k + 1) * TILE, :]
                nc.sync.dma_start(dst_ap, stage[:, :tw_next])

            if not last and "noag" not in ablate:
                nc.gpsimd.collective_compute(
                    "AllGather", mybir.AluOpType.bypass,
                    replica_groups=[list(range(NC))],
                    ins=[shards[li].ap()], outs=[tables[li + 1].ap()])
            elif not last:
                nc.sync.dma_start(tables[li + 1][0:npc, :], shards[li][0:npc, :])

    return nc


# ----------------------------------------------------------------------------
# Entry point
# ----------------------------------------------------------------------------

def kernel(**inputs):
    x = np.asarray(inputs["x"], dtype=np.float32)
    edge_index = np.asarray(inputs["edge_index"])
    edge_weight = np.asarray(inputs["edge_weight"], dtype=np.float32)
    widths = []
    i = 1
    while f"W{i}" in inputs:
        widths.append(np.asarray(inputs[f"W{i}"]).shape[1])
        i += 1

    P = build_plan(x, edge_index, edge_weight, widths)

    weights = [np.asarray(inputs[f"W{i+1}"], dtype=np.float32) for i in range(len(widths))]
    gammas = [np.asarray(inputs[f"g{i+1}"], dtype=np.float32) for i in range(len(widths))]
    betas = [np.asarray(inputs[f"bb{i+1}"], dtype=np.float32) for i in range(len(widths))]
    # biases b{i} are mathematically cancelled by BN mean subtraction; omitted.

    nc = bacc.Bacc("TRN2", target_bir_lowering=False, debug=False, num_devices=NC)
    build_graph(nc, P, weights, gammas, betas)
    nc.compile()

    # input images
    def wimg(W):
        cin, cout = W.shape
        kcs = math.ceil(cin / TILE)
        img = np.zeros((TILE, kcs * cout), dtype=np.float32)
        for kc in range(kcs):
            w = min(TILE, cin - kc * TILE)
            img[:w, kc * cout:(kc + 1) * cout] = W[kc * TILE: kc * TILE + w]
        return img

    def fvec(v):
        cout = v.shape[0]
        hvs = math.ceil(cout / TILE)
        img = np.zeros((TILE, hvs), dtype=np.float32)
        for h in range(hvs):
            hw = min(TILE, cout - h * TILE)
            img[:hw, h] = v[h * TILE: h * TILE + hw]
        return img

    ident = np.eye(TILE, dtype=ml_dtypes.bfloat16)
    identf = np.eye(TILE, dtype=np.float32)

    in_maps = []
    for c in range(NC):
        m = {
            "idx": P.idx_flats[c],
            "sblk": P.s_flats[c],
            "maskdinv": P.maskdinv[c],
            "h0": P.h0,
            "ident": ident,
            "identf": identf,
        }
        for li in range(len(widths)):
            m[f"W{li}"] = wimg(weights[li])
            m[f"g{li}"] = fvec(gammas[li])
            m[f"bb{li}"] = fvec(betas[li])
        in_maps.append(m)

    results, times = _run_pjrt(nc, in_maps,
                               trials=int(os.environ.get("GCN_TRIALS", "1")))
    kernel.last_times = times

    out_dev = np.concatenate([results[c]["out"] for c in range(NC)], axis=0)
    out = np.empty((P.N, widths[-1]), dtype=np.float32)
    real = P.orig_of_dev >= 0
    out[P.orig_of_dev[real]] = out_dev[real]
    return out


def measure_floor(trials=6):
    """Null 2-DMA kernel through the same runner: axon dispatch floor."""
    import concourse.bacc as bacc_
    import concourse.tile as tile_
    nc = bacc_.Bacc("TRN2", target_bir_lowering=False, debug=False, num_devices=NC)
    inp = nc.dram_tensor("a", [128, 128], F32, kind="ExternalInput")
    out = nc.dram_tensor("out", [128, 128], F32, kind="ExternalOutput")
    with tile_.TileContext(nc) as tc:
        with tc.tile_pool(name="p", bufs=1) as pool:
            t = pool.tile([128, 128], F32)
            nc.sync.dma_start(t[:], inp[:, :])
            nc.sync.dma_start(out[:, :], t[:])
    nc.compile()
    in_maps = [{"a": np.ones((128, 128), np.float32)} for _ in range(NC)]
    _, times = _run_pjrt(nc, in_maps, trials=trials)
    return times


def _run_pjrt(nc, in_maps, trials=1):
    """Replicates bass2jax.run_bass_via_pjrt but with device-staged inputs and
    wall-clock timing of repeated executions."""
    import time
    import jax
    from jax.sharding import Mesh, PartitionSpec, NamedSharding
    from jax.experimental.shard_map import shard_map
    import concourse.bass2jax as b2j
    import concourse.mybir as mb

    b2j.install_neuronx_cc_hook()
    n_cores = NC
    partition_name = nc.partition_id_tensor.name if nc.partition_id_tensor else None
    in_names, out_names, out_avals, zero_outs = [], [], [], []
    for alloc in nc.m.functions[0].allocations:
        if not isinstance(mb.MemoryLocationSet, type) or not isinstance(alloc, mb.MemoryLocationSet):
            continue
        if not alloc.memorylocations:
            continue
        name = alloc.memorylocations[0].name
        if alloc.kind == "ExternalInput":
            if name != partition_name:
                in_names.append(name)
        elif alloc.kind == "ExternalOutput":
            out_names.append(name)
            shape = tuple(alloc.tensor_shape)
            dtype = mb.dt.np(alloc.dtype)
            out_avals.append(jax.core.ShapedArray(shape, dtype))
            zero_outs.append(np.zeros(shape, dtype))
    n_params = len(in_names)
    n_outs = len(out_avals)
    in_names_all = list(in_names) + list(out_names)
    if partition_name is not None:
        in_names_all.append(partition_name)
    donate = tuple(range(n_params, n_params + n_outs))

    def _body(*args):
        operands = list(args)
        if partition_name is not None:
            operands.append(b2j.partition_id_tensor())
        outs = b2j._bass_exec_p.bind(
            *operands,
            out_avals=tuple(out_avals),
            in_names=tuple(in_names_all),
            out_names=tuple(out_names),
            lowering_input_output_aliases=(),
            sim_require_finite=True,
            sim_require_nnan=True,
            nc=nc,
        )
        return tuple(outs)

    devices = jax.devices()[:n_cores]
    mesh = Mesh(np.asarray(devices), ("core",))
    in_specs = (PartitionSpec("core"),) * (n_params + n_outs)
    out_specs = (PartitionSpec("core"),) * len(out_names)
    sharded = jax.jit(
        shard_map(_body, mesh=mesh, in_specs=in_specs, out_specs=out_specs,
                  check_rep=False),
        donate_argnums=donate, keep_unused=True)
    shd = NamedSharding(mesh, PartitionSpec("core"))
    concat_in = [
        jax.device_put(
            np.concatenate([np.asarray(in_maps[c][nm]) for c in range(n_cores)], axis=0),
            shd)
        for nm in in_names
    ]
    jax.block_until_ready(concat_in)

    times = []
    out_arrs = None
    for t in range(max(1, trials)):
        dev_zeros = [
            jax.device_put(np.zeros((n_cores * z.shape[0], *z.shape[1:]), z.dtype), shd)
            for z in zero_outs
        ]
        jax.block_until_ready(dev_zeros)
        t0 = time.perf_counter()
        out_arrs = sharded(*concat_in, *dev_zeros)
        jax.block_until_ready(out_arrs)
        times.append(time.perf_counter() - t0)

    results = [
        {name: np.asarray(out_arrs[i]).reshape(n_cores, *out_avals[i].shape)[c]
         for i, name in enumerate(out_names)}
        for c in range(n_cores)
    ]
    return results, times
